# revision 5
# baseline (speedup 1.0000x reference)
"""Trainium2 Bass kernel for nn_AttentionAggregator2 (gnn_message_passing).

Per-core computation (data-parallel over nodes, 8 cores):
  ncat = [neibs | edge]                       [rows=65536, 320]
  pre1 = tanh(ncat @ att2_w1.T)               [rows, 64]
  z    = (tanh(x @ att_w1.T) @ att_w2.T) @ att2_w2      [nodes, 64]
  s    = (pre1 . z[node]) / 8                 per row
  w    = exp(s) * exp(-9999999*mask)          (softmax without max-subtraction;
                                               normalization folded into den)
  num  = sum_k w * ncat ; den = sum_k w       per node
  out  = relu([x @ fcx_w.T | (num/den) @ fcn_w.T])

Layout strategy:
  - neibs/edge are cast-loaded f32->bf16 (SWDGE) into a "natural" tile
    [128 rows x (subtile, 384)] (256 neibs | 64 edge | 1 ones | pad).
  - The feature-major ("transposed") layout needed by the pre1 matmul is
    produced on-chip with the DMA xbar transpose (bf16, HWDGE on scalar) --
    no second HBM read and no big DVE/ACT pass.
  - Scores are computed as a pairs matmul (tanh_pre1^T as stationary,
    z^T as moving), exp'd on ACT during PSUM eviction, masked into
    block-diagonal ws on DVE (one scalar_tensor_tensor op), and fed back
    into the aggregation matmul as the stationary operand, with a ones
    column producing den for free.
"""

import sys
import numpy as np

for _p in ("/opt/trn_rl_repo",):
    if _p not in sys.path:
        sys.path.insert(0, _p)

import concourse.bacc as bacc
import concourse.bass as bass
import concourse.mybir as mybir
import concourse.tile as tile
from concourse.bass_utils import run_bass_kernel_spmd

# Problem dims (hardcoded: nn_AttentionAggregator2_46033459479180)
N, K, D, E, H, O = 16384, 32, 256, 64, 64, 256
NCORES = 8
NL = N // NCORES            # 2048 nodes per core
ROWS = NL * K               # 65536 rows per core
DPE = D + E                 # 320
BLK = 384                   # natural block: 256 neibs | 64 edge | 1 ones | 63 pad
NB = 16                     # 128-row subtiles per load group
GROUP_ROWS = NB * 128       # 2048 rows
NG = ROWS // GROUP_ROWS     # 32 load groups per core

F32 = mybir.dt.float32
BF = mybir.dt.bfloat16
AF = mybir.ActivationFunctionType
MULT = mybir.AluOpType.mult

INV_SQRT_H = 1.0 / np.sqrt(H).astype(np.float32)
MASK_SCALE = -9999999.0


def build_kernel(nc, ng=NG):
    """Emit the per-core kernel. ng<NG builds a row-prefix (for simulation)."""
    x = nc.dram_tensor("x", [NL, D], F32, kind="ExternalInput").ap()
    neibs = nc.dram_tensor("neibs", [ROWS, D], F32, kind="ExternalInput").ap()
    edge = nc.dram_tensor("edge_emb", [ROWS, E], F32, kind="ExternalInput").ap()
    mask = nc.dram_tensor("mask", [NL, K], F32, kind="ExternalInput").ap()
    att_w1 = nc.dram_tensor("att_w1", [H, D], F32, kind="ExternalInput").ap()
    att_w2 = nc.dram_tensor("att_w2", [H, H], F32, kind="ExternalInput").ap()
    att2_w1 = nc.dram_tensor("att2_w1", [H, DPE], F32, kind="ExternalInput").ap()
    att2_w2 = nc.dram_tensor("att2_w2", [H, H], F32, kind="ExternalInput").ap()
    fcx_w = nc.dram_tensor("fcx_w", [O, D], F32, kind="ExternalInput").ap()
    fcn_w = nc.dram_tensor("fcn_w", [O, DPE], F32, kind="ExternalInput").ap()
    out = nc.dram_tensor("out", [NL, 2 * O], F32, kind="ExternalOutput").ap()

    # DRAM views
    neibs_g = neibs.rearrange("(g s p) d -> g p s d", p=128, s=NB)   # row = g*2048+s*128+p
    edge_g = edge.rearrange("(g s p) e -> g p s e", p=128, s=NB)
    mask_pc = mask.rearrange("(p a) k -> p (a k)", p=128)            # [128, 512]; col c of part p = row 512p+c
    x_sp = x.rearrange("(s p) d -> p s d", p=128)                    # [128, 16, 256]
    fcx_rp = fcx_w.rearrange("(r p) d -> p r d", p=128)              # [128, 2, 256]
    fcn_rp = fcn_w.rearrange("(r p) d -> p r d", p=128)              # [128, 2, 320]
    out_b = out.rearrange("(b p) o -> b p o", p=128)                 # [16, 128, 512]

    n_node_blocks = (ng * GROUP_ROWS) // (K * 128)  # 128-node output blocks

    with tile.TileContext(nc) as tc:
        with (
            tc.tile_pool(name="consts", bufs=1) as consts,
            tc.tile_pool(name="natp", bufs=3) as natp,
            tc.tile_pool(name="edgefp", bufs=3) as edgefp,
            tc.tile_pool(name="ttp", bufs=3) as ttp,
            tc.tile_pool(name="tanhp", bufs=8) as tanhp,
            tc.tile_pool(name="expp", bufs=4) as exppl,
            tc.tile_pool(name="wsp", bufs=16) as wsp,
            tc.tile_pool(name="aggnp", bufs=2) as aggnp,
            tc.tile_pool(name="recp", bufs=2) as recp,
            tc.tile_pool(name="outsbp", bufs=3) as outsbp,
            tc.tile_pool(name="psum_pre", bufs=2, space="PSUM") as psum_pre,
            tc.tile_pool(name="psum_pairs", bufs=2, space="PSUM") as psum_pairs,
            tc.tile_pool(name="psum_agg", bufs=2, space="PSUM") as psum_agg,
            tc.tile_pool(name="psum_out", bufs=2, space="PSUM") as psum_out,
        ):
            # ---------------- weights & constants ----------------
            aw1_nat = consts.tile([H, D], BF, tag="aw1n")
            nc.gpsimd.dma_start(out=aw1_nat[:], in_=att_w1)
            aw2_nat = consts.tile([H, 128], BF, tag="aw2n")
            nc.vector.memset(aw2_nat[:], 0)
            nc.gpsimd.dma_start(out=aw2_nat[:, 0:H], in_=att_w2)
            a2w1_nat = consts.tile([H, BLK], BF, tag="a2w1n")
            nc.vector.memset(a2w1_nat[:], 0)
            nc.gpsimd.dma_start(out=a2w1_nat[:, 0:DPE], in_=att2_w1)
            a2w2_nat = consts.tile([H, H], BF, tag="a2w2n")
            nc.gpsimd.dma_start(out=a2w2_nat[:], in_=att2_w2)
            fcx_nat = consts.tile([128, 2, D], BF, tag="fcxn")
            nc.gpsimd.dma_start(out=fcx_nat[:], in_=fcx_rp)
            fcn_nat = consts.tile([128, 2, BLK], BF, tag="fcnn")
            nc.vector.memset(fcn_nat[:], 0)
            nc.gpsimd.dma_start(out=fcn_nat[:, :, 0:DPE], in_=fcn_rp)

            # transposed weights (xbar; contiguous dests)
            a2w1T = consts.tile([128, 3, H], BF, tag="a2w1T")  # [d', c, h]
            nc.scalar.dma_start(out=a2w1T[:], in_=a2w1_nat[:], transpose=True)
            aw1T = consts.tile([128, 2, H], BF, tag="aw1T")
            nc.scalar.dma_start(out=aw1T[:], in_=aw1_nat[:], transpose=True)
            aw2T = consts.tile([128, 1, H], BF, tag="aw2T")
            nc.scalar.dma_start(out=aw2T[:], in_=aw2_nat[:], transpose=True)
            fcxT = []
            for r in range(2):
                t = consts.tile([128, 2, 128], BF, tag=f"fcxT{r}")  # [d', c, o']
                nc.scalar.dma_start(out=t[:], in_=fcx_nat[:, r, :], transpose=True)
                fcxT.append(t)
            fcnT = []
            for r in range(2):
                t = consts.tile([128, 3, 128], BF, tag=f"fcnT{r}")
                nc.scalar.dma_start(out=t[:], in_=fcn_nat[:, r, :], transpose=True)
                fcnT.append(t)

            # sel masks: sel8[p, v, j] = 1 iff j == 4v + p//32
            sel8 = consts.tile([128, 8, K], BF, tag="sel8")
            nc.vector.memset(sel8[:], 0)
            for v in range(8):
                for b in range(4):
                    j = 4 * v + b
                    nc.vector.memset(sel8[32 * b:32 * b + 32, v, j:j + 1], 1.0)

            # emask = exp(-9999999*mask), transposed to row-major-by-subtile
            maskpc = consts.tile([128, 512], F32, tag="maskpc")
            nc.sync.dma_start(out=maskpc[:], in_=mask_pc)
            emask_pc = consts.tile([128, 512], BF, tag="emaskpc")
            nc.scalar.activation(emask_pc[:], maskpc[:], AF.Exp, scale=MASK_SCALE)
            emT = consts.tile([128, 4, 128], BF, tag="emT")  # [:, S%4, S//4] = rows of subtile S
            nc.scalar.dma_start(out=emT[:], in_=emask_pc[:], transpose=True)

            # ---------------- x path: x_T, z_T ----------------
            x_nat = consts.tile([128, 16, D], BF, tag="xnat")
            nc.gpsimd.dma_start(out=x_nat[:], in_=x_sp)
            xT = consts.tile([128, 32, 128], BF, tag="xT")  # block jj=2s+c
            nc.scalar.dma_start(out=xT[:], in_=x_nat[:], transpose=True)
            xT_sc = xT[:].rearrange("p (s c) f -> p s c f", c=2)

            hidtanh = consts.tile([H, NL], BF, tag="hidtanh")
            xatt = consts.tile([H, NL], BF, tag="xatt")
            zT = consts.tile([H, NL], BF, tag="zT")
            for sg in range(4):
                nsl = slice(512 * sg, 512 * (sg + 1))
                hid_ps = psum_pre.tile([H, 512], F32, tag="pre")
                for c in range(2):
                    nc.tensor.matmul(
                        hid_ps[:], lhsT=aw1T[:, c, :],
                        rhs=xT_sc[:, 4 * sg:4 * sg + 4, c, :],
                        start=(c == 0), stop=(c == 1))
                nc.scalar.activation(hidtanh[:, nsl], hid_ps[:], AF.Tanh)
            for sg in range(4):
                nsl = slice(512 * sg, 512 * (sg + 1))
                xa_ps = psum_pre.tile([H, 512], F32, tag="pre")
                nc.tensor.matmul(xa_ps[:], lhsT=aw2T[0:H, 0, :], rhs=hidtanh[:, nsl],
                                 start=True, stop=True)
                nc.vector.tensor_copy(out=xatt[:, nsl], in_=xa_ps[:])
            for sg in range(4):
                nsl = slice(512 * sg, 512 * (sg + 1))
                z_ps = psum_pre.tile([H, 512], F32, tag="pre")
                nc.tensor.matmul(z_ps[:], lhsT=a2w2_nat[:], rhs=xatt[:, nsl],
                                 start=True, stop=True)
                nc.vector.tensor_copy(out=zT[:, nsl], in_=z_ps[:])

            # ---------------- main loop ----------------
            for gi2 in range(n_node_blocks):
                agg_ps = psum_agg.tile([128, DPE + 1], F32, tag="agg")
                for gsub in range(2):
                    gi = 2 * gi2 + gsub
                    # load group: natural bf16 tile + xbar transpose
                    nat = natp.tile([128, NB, BLK], BF, tag="nat")
                    nc.gpsimd.dma_start(out=nat[:, :, 0:D], in_=neibs_g[gi])
                    edgef = edgefp.tile([128, NB, E], F32, tag="edgef")
                    nc.sync.dma_start(out=edgef[:], in_=edge_g[gi])
                    nc.vector.tensor_copy(out=nat[:, :, D:DPE], in_=edgef[:])
                    nc.vector.memset(nat[:, :, DPE:DPE + 1], 1.0)
                    nc.vector.memset(nat[:, :, DPE + 1:BLK], 0)
                    tT = ttp.tile([128, 3 * NB, 128], BF, tag="tT")
                    nc.scalar.dma_start(out=tT[:], in_=nat[:], transpose=True)
                    tT_sc = tT[:].rearrange("p (s c) f -> p s c f", c=3)

                    # pre1^T = att2_w1 @ ncat^T, tanh'd on eviction
                    tanh_tiles = []
                    for sg in range(4):
                        pre_ps = psum_pre.tile([H, 512], F32, tag="pre")
                        for c in range(2):
                            nc.tensor.matmul(
                                pre_ps[:], lhsT=a2w1T[:, c, :],
                                rhs=tT_sc[:, 4 * sg:4 * sg + 4, c, :],
                                start=(c == 0), stop=False)
                        for si in range(4):
                            s = 4 * sg + si
                            nc.tensor.matmul(
                                pre_ps[:, 128 * si:128 * si + 128],
                                lhsT=a2w1T[0:H, 2, :], rhs=tT[0:H, 3 * s + 2, :],
                                start=False, stop=(si == 3))
                        ttile = tanhp.tile([H, 512], BF, tag="tanhT")
                        nc.scalar.activation(ttile[:], pre_ps[:], AF.Tanh)
                        tanh_tiles.append(ttile)

                    # pairs scores -> exp -> ws (block-diag), aggregation
                    for gh in range(2):
                        nodes0 = gi * 64 + 32 * gh
                        pairs_ps = psum_pairs.tile([128, 256], F32, tag="pairs")
                        for si8 in range(8):
                            s = 8 * gh + si8
                            nc.tensor.matmul(
                                pairs_ps[:, 32 * si8:32 * si8 + 32],
                                lhsT=tanh_tiles[s // 4][:, 128 * (s % 4):128 * (s % 4) + 128],
                                rhs=zT[:, nodes0:nodes0 + 32],
                                start=True, stop=True)
                        expt = exppl.tile([128, 256], BF, tag="exp")
                        nc.scalar.activation(expt[:], pairs_ps[:], AF.Exp, scale=INV_SQRT_H)
                        strip = (2 * gi + gh) % 4
                        for si8 in range(8):
                            s = 8 * gh + si8
                            S = gi * NB + s
                            ws = wsp.tile([128, K], BF, tag="ws")
                            nc.vector.scalar_tensor_tensor(
                                out=ws[:], in0=expt[:, 32 * si8:32 * si8 + 32],
                                scalar=emT[:, S % 4, (S // 4):(S // 4) + 1],
                                in1=sel8[:, si8, :], op0=MULT, op1=MULT)
                            nc.tensor.matmul(
                                agg_ps[32 * strip:32 * strip + 32, :],
                                lhsT=ws[:], rhs=nat[:, s, 0:DPE + 1],
                                start=(si8 == 0), stop=(si8 == 7),
                                tile_position=(0, 32 * strip))

                # normalize, transpose agg, output projections
                rec = recp.tile([128, 1], F32, tag="rec")
                nc.vector.reciprocal(rec[:], agg_ps[:, DPE:DPE + 1])
                agg_nat = aggnp.tile([128, BLK], BF, tag="aggn")
                nc.vector.tensor_scalar_mul(agg_nat[:, 0:DPE], agg_ps[:, 0:DPE], rec[:])
                nc.vector.memset(agg_nat[:, DPE:BLK], 0)
                aggT = aggnp.tile([128, 3, 128], BF, tag="aggT")
                nc.scalar.dma_start(out=aggT[:], in_=agg_nat[:], transpose=True)

                out2_ps = psum_out.tile([128, 2 * 128], F32, tag="outp")
                for r in range(2):
                    for c in range(2):
                        nc.tensor.matmul(
                            out2_ps[:, 128 * r:128 * r + 128],
                            lhsT=aggT[:, c, :], rhs=fcnT[r][:, c, :],
                            start=(c == 0), stop=False)
                    nc.tensor.matmul(
                        out2_ps[:, 128 * r:128 * r + 128],
                        lhsT=aggT[0:H, 2, :], rhs=fcnT[r][0:H, 2, :],
                        start=False, stop=True)
                out1_ps = psum_out.tile([128, 2 * 128], F32, tag="outp")
                for r in range(2):
                    for c in range(2):
                        nc.tensor.matmul(
                            out1_ps[:, 128 * r:128 * r + 128],
                            lhsT=xT[:, 2 * gi2 + c, :], rhs=fcxT[r][:, c, :],
                            start=(c == 0), stop=(c == 1))
                outsb = outsbp.tile([128, 2 * O], F32, tag="outsb")
                nc.scalar.activation(outsb[:, 0:O], out1_ps[:], AF.Relu)
                nc.scalar.activation(outsb[:, O:2 * O], out2_ps[:], AF.Relu)
                nc.sync.dma_start(out=out_b[gi2], in_=outsb[:])

    return nc


_CACHE = {}


def _get_compiled():
    if "nc" not in _CACHE:
        nc = bacc.Bacc("TRN2", target_bir_lowering=False, debug=False)
        build_kernel(nc)
        nc.compile()
        _CACHE["nc"] = nc
    return _CACHE["nc"]


def make_in_maps(x, neibs, edge_emb, mask, att_w1, att_w2, att2_w1, att2_w2,
                 fcx_w, fcn_w):
    x = np.ascontiguousarray(np.asarray(x, np.float32))
    neibs = np.ascontiguousarray(np.asarray(neibs, np.float32))
    edge_emb = np.ascontiguousarray(np.asarray(edge_emb, np.float32))
    mask = np.ascontiguousarray(np.asarray(mask, np.float32))
    w = {
        "att_w1": np.ascontiguousarray(np.asarray(att_w1, np.float32)),
        "att_w2": np.ascontiguousarray(np.asarray(att_w2, np.float32)),
        "att2_w1": np.ascontiguousarray(np.asarray(att2_w1, np.float32)),
        "att2_w2": np.ascontiguousarray(np.asarray(att2_w2, np.float32)),
        "fcx_w": np.ascontiguousarray(np.asarray(fcx_w, np.float32)),
        "fcn_w": np.ascontiguousarray(np.asarray(fcn_w, np.float32)),
    }
    in_maps = []
    for i in range(NCORES):
        in_maps.append({
            "x": x[i * NL:(i + 1) * NL],
            "neibs": neibs[i * ROWS:(i + 1) * ROWS],
            "edge_emb": edge_emb[i * ROWS:(i + 1) * ROWS],
            "mask": mask[i * NL:(i + 1) * NL],
            **w,
        })
    return in_maps


def run(in_maps, trace=False):
    nc = _get_compiled()
    return run_bass_kernel_spmd(nc, in_maps, list(range(NCORES)), trace=trace)


def kernel(x, neibs, edge_emb, mask, att_w1, att_w2, att2_w1, att2_w2,
           fcx_w, fcn_w):
    in_maps = make_in_maps(x, neibs, edge_emb, mask, att_w1, att_w2,
                           att2_w1, att2_w2, fcx_w, fcn_w)
    res = run(in_maps)
    outs = [np.asarray(res.results[i]["out"], np.float32) for i in range(NCORES)]
    return np.concatenate(outs, axis=0)


# revision 27
# speedup vs baseline: 1.4221x; 1.4221x over previous
"""Trainium2 Bass kernel for nn_AttentionAggregator2 (gnn_message_passing).

Per-core computation (data-parallel over nodes, 8 cores):
  ncat = [neibs | edge]                       [rows=65536, 320]
  pre1 = tanh(ncat @ att2_w1.T)               [rows, 64]
  z    = (tanh(x @ att_w1.T) @ att_w2.T) @ att2_w2      [nodes, 64]
  s    = (pre1 . z[node]) / 8                 per row
  w    = exp(s) * exp(-9999999*mask)          (softmax without max-subtraction;
                                               normalization folded into den)
  num  = sum_k w * ncat ; den = sum_k w       per node
  out  = relu([x @ fcx_w.T | (num/den) @ fcn_w.T])

Layout strategy:
  - neibs/edge are cast-loaded f32->bf16 (SWDGE) into a "natural" tile
    [128 rows x (subtile, 384)] (256 neibs | 64 edge | 1 ones | pad).
  - The feature-major ("transposed") layout needed by the pre1 matmul is
    produced on-chip with the DMA xbar transpose (bf16, HWDGE on scalar) --
    no second HBM read and no big DVE/ACT pass.
  - Scores are computed as a pairs matmul (tanh_pre1^T as stationary,
    z^T as moving), exp'd on ACT during PSUM eviction, masked into
    block-diagonal ws on DVE (one scalar_tensor_tensor op), and fed back
    into the aggregation matmul as the stationary operand, with a ones
    column producing den for free.
"""

import sys
import numpy as np

for _p in ("/opt/trn_rl_repo",):
    if _p not in sys.path:
        sys.path.insert(0, _p)

import concourse.bacc as bacc
import concourse.bass as bass
import concourse.mybir as mybir
import concourse.tile as tile
from concourse.bass_utils import run_bass_kernel_spmd

# Problem dims (hardcoded: nn_AttentionAggregator2_46033459479180)
N, K, D, E, H, O = 16384, 32, 256, 64, 64, 256
NCORES = 8
NL = N // NCORES            # 2048 nodes per core
ROWS = NL * K               # 65536 rows per core
DPE = D + E                 # 320
BLK = 384                   # natural block: 256 neibs | 64 edge | 1 ones | 63 pad
NB = 16                     # 128-row subtiles per load group
GROUP_ROWS = NB * 128       # 2048 rows
NG = ROWS // GROUP_ROWS     # 32 load groups per core

F32 = mybir.dt.float32
BF = mybir.dt.bfloat16
AF = mybir.ActivationFunctionType
MULT = mybir.AluOpType.mult

INV_SQRT_H = 1.0 / np.sqrt(H).astype(np.float32)
MASK_SCALE = -9999999.0


def build_kernel(nc, ng=NG, init_pad=False):
    """Emit the per-core kernel. ng<NG builds a row-prefix (for simulation).

    init_pad=True memsets the junk pad columns so CoreSim's uninitialized-read
    checker stays quiet; the pad is never consumed, so hardware builds skip it.
    """
    x = nc.dram_tensor("x", [NL, D], F32, kind="ExternalInput").ap()
    neibs = nc.dram_tensor("neibs", [ROWS, D], F32, kind="ExternalInput").ap()
    edge = nc.dram_tensor("edge_emb", [ROWS, E], F32, kind="ExternalInput").ap()
    mask = nc.dram_tensor("mask", [NL, K], F32, kind="ExternalInput").ap()
    att_w1 = nc.dram_tensor("att_w1", [H, D], F32, kind="ExternalInput").ap()
    att_w2 = nc.dram_tensor("att_w2", [H, H], F32, kind="ExternalInput").ap()
    att2_w1 = nc.dram_tensor("att2_w1", [H, DPE], F32, kind="ExternalInput").ap()
    att2_w2 = nc.dram_tensor("att2_w2", [H, H], F32, kind="ExternalInput").ap()
    fcx_w = nc.dram_tensor("fcx_w", [O, D], F32, kind="ExternalInput").ap()
    fcn_w = nc.dram_tensor("fcn_w", [O, DPE], F32, kind="ExternalInput").ap()
    out = nc.dram_tensor("out", [NL, 2 * O], F32, kind="ExternalOutput").ap()

    # DRAM views
    neibs_g = neibs.rearrange("(g s p) d -> g p s d", p=128, s=NB)   # row = g*2048+s*128+p
    edge_g = edge.rearrange("(g s p) e -> g p s e", p=128, s=NB)
    mask_pc = mask.rearrange("(p a) k -> p (a k)", p=128)            # [128, 512]; col c of part p = row 512p+c
    x_sp = x.rearrange("(s p) d -> p s d", p=128)                    # [128, 16, 256]
    fcx_rp = fcx_w.rearrange("(r p) d -> p r d", p=128)              # [128, 2, 256]
    fcn_rp = fcn_w.rearrange("(r p) d -> p r d", p=128)              # [128, 2, 320]
    out_b = out.rearrange("(b p) o -> b p o", p=128)                 # [16, 128, 512]

    n_node_blocks = (ng * GROUP_ROWS) // (K * 128)  # 128-node output blocks

    with tile.TileContext(nc) as tc:
        with (
            tc.tile_pool(name="consts", bufs=1) as consts,
            tc.tile_pool(name="natp", bufs=7) as natp,
            tc.tile_pool(name="edgefp", bufs=3) as edgefp,
            tc.tile_pool(name="ttp", bufs=4) as ttp,
            tc.tile_pool(name="tanhp", bufs=8) as tanhp,
            tc.tile_pool(name="expp", bufs=4) as exppl,
            tc.tile_pool(name="wsp", bufs=16) as wsp,
            tc.tile_pool(name="aggnp", bufs=2) as aggnp,
            tc.tile_pool(name="recp", bufs=2) as recp,
            tc.tile_pool(name="outsbp", bufs=3) as outsbp,
            tc.tile_pool(name="psum_pre", bufs=2, space="PSUM") as psum_pre,
            tc.tile_pool(name="psum_pairs", bufs=2, space="PSUM") as psum_pairs,
            tc.tile_pool(name="psum_agg", bufs=2, space="PSUM") as psum_agg,
            tc.tile_pool(name="psum_out", bufs=2, space="PSUM") as psum_out,
        ):
            # ---------------- weights & constants ----------------
            aw1_nat = consts.tile([H, D], BF, tag="aw1n")
            nc.gpsimd.dma_start(out=aw1_nat[:], in_=att_w1)
            aw2_nat = consts.tile([H, 128], BF, tag="aw2n")
            nc.vector.memset(aw2_nat[:], 0)
            nc.gpsimd.dma_start(out=aw2_nat[:, 0:H], in_=att_w2)
            # cols 320:384 replicate the edge block so the transposed chunk 2
            # carries W1b^T on partitions 64-127 too (for hi-half edge matmuls)
            a2w1_nat = consts.tile([H, BLK], BF, tag="a2w1n")
            nc.gpsimd.dma_start(out=a2w1_nat[:, 0:DPE], in_=att2_w1)
            nc.gpsimd.dma_start(out=a2w1_nat[:, DPE:BLK], in_=att2_w1[:, D:DPE])
            a2w2_nat = consts.tile([H, H], BF, tag="a2w2n")
            nc.gpsimd.dma_start(out=a2w2_nat[:], in_=att2_w2)
            fcx_nat = consts.tile([128, 2, D], BF, tag="fcxn")
            nc.gpsimd.dma_start(out=fcx_nat[:], in_=fcx_rp)
            fcn_nat = consts.tile([128, 2, BLK], BF, tag="fcnn")
            nc.vector.memset(fcn_nat[:], 0)
            nc.gpsimd.dma_start(out=fcn_nat[:, :, 0:DPE], in_=fcn_rp)

            # transposed weights (xbar; contiguous dests)
            a2w1T = consts.tile([128, 3, H], BF, tag="a2w1T")  # [d', c, h]
            nc.scalar.dma_start(out=a2w1T[:], in_=a2w1_nat[:], transpose=True)
            aw1T = consts.tile([128, 2, H], BF, tag="aw1T")
            nc.scalar.dma_start(out=aw1T[:], in_=aw1_nat[:], transpose=True)
            aw2T = consts.tile([128, 1, H], BF, tag="aw2T")
            nc.scalar.dma_start(out=aw2T[:], in_=aw2_nat[:], transpose=True)
            fcxT = []
            for r in range(2):
                t = consts.tile([128, 2, 128], BF, tag=f"fcxT{r}")  # [d', c, o']
                nc.scalar.dma_start(out=t[:], in_=fcx_nat[:, r, :], transpose=True)
                fcxT.append(t)
            fcnT = []
            for r in range(2):
                t = consts.tile([128, 3, 128], BF, tag=f"fcnT{r}")
                nc.scalar.dma_start(out=t[:], in_=fcn_nat[:, r, :], transpose=True)
                fcnT.append(t)

            ones_col = consts.tile([128, 1], BF, tag="ones")
            nc.vector.memset(ones_col[:], 1.0)

            # sel masks: sel8[p, v, j] = 1 iff j == 4v + p//32
            sel8 = consts.tile([128, 8, K], BF, tag="sel8")
            nc.vector.memset(sel8[:], 0)
            for v in range(8):
                for b in range(4):
                    j = 4 * v + b
                    nc.vector.memset(sel8[32 * b:32 * b + 32, v, j:j + 1], 1.0)

            # emask = exp(-9999999*mask), transposed to row-major-by-subtile
            maskpc = consts.tile([128, 512], F32, tag="maskpc")
            nc.sync.dma_start(out=maskpc[:], in_=mask_pc)
            emask_pc = consts.tile([128, 512], BF, tag="emaskpc")
            nc.scalar.activation(emask_pc[:], maskpc[:], AF.Exp, scale=MASK_SCALE)
            emT = consts.tile([128, 4, 128], BF, tag="emT")  # [:, S%4, S//4] = rows of subtile S
            nc.scalar.dma_start(out=emT[:], in_=emask_pc[:], transpose=True)

            # ---------------- x path: x_T, z_T ----------------
            x_nat = consts.tile([128, 16, D], BF, tag="xnat")
            nc.gpsimd.dma_start(out=x_nat[:], in_=x_sp)
            xT = consts.tile([128, 32, 128], BF, tag="xT")  # block jj=2s+c
            nc.scalar.dma_start(out=xT[:], in_=x_nat[:], transpose=True)
            xT_sc = xT[:].rearrange("p (s c) f -> p s c f", c=2)

            hidtanh = consts.tile([H, NL], BF, tag="hidtanh")
            xatt = consts.tile([H, NL], BF, tag="xatt")
            zT = consts.tile([H, NL], BF, tag="zT")
            for sg in range(4):
                nsl = slice(512 * sg, 512 * (sg + 1))
                hid_ps = psum_pre.tile([H, 512], F32, tag="pre")
                for c in range(2):
                    nc.tensor.matmul(
                        hid_ps[:], lhsT=aw1T[:, c, :],
                        rhs=xT_sc[:, 4 * sg:4 * sg + 4, c, :],
                        start=(c == 0), stop=(c == 1))
                nc.scalar.activation(hidtanh[:, nsl], hid_ps[:], AF.Tanh)
            for sg in range(4):
                nsl = slice(512 * sg, 512 * (sg + 1))
                xa_ps = psum_pre.tile([H, 512], F32, tag="pre")
                nc.tensor.matmul(xa_ps[:], lhsT=aw2T[0:H, 0, :], rhs=hidtanh[:, nsl],
                                 start=True, stop=True)
                nc.vector.tensor_copy(out=xatt[:, nsl], in_=xa_ps[:])
            for sg in range(4):
                nsl = slice(512 * sg, 512 * (sg + 1))
                z_ps = psum_pre.tile([H, 512], F32, tag="pre")
                nc.tensor.matmul(z_ps[:], lhsT=a2w2_nat[:], rhs=xatt[:, nsl],
                                 start=True, stop=True)
                nc.vector.tensor_copy(out=zT[:, nsl], in_=z_ps[:])

            # ---------------- main loop ----------------
            def emit_projections(gi2, agg_ps):
                # normalize, transpose agg, output projections (called one
                # node-block late so PE never stalls on the aggT xbar hop)
                rec = recp.tile([128, 1], F32, tag="rec")
                nc.vector.reciprocal(rec[:], agg_ps[:, DPE:DPE + 1])
                agg_nat = aggnp.tile([128, BLK], BF, tag="aggn")
                nc.vector.tensor_scalar_mul(agg_nat[:, 0:DPE], agg_ps[:, 0:DPE], rec[:])
                nc.vector.memset(agg_nat[:, DPE:BLK], 0)
                aggT = aggnp.tile([128, 3, 128], BF, tag="aggT")
                nc.scalar.dma_start(out=aggT[:], in_=agg_nat[:], transpose=True)

                out2_ps = psum_out.tile([128, 2 * 128], F32, tag="outp")
                for r in range(2):
                    for c in range(2):
                        nc.tensor.matmul(
                            out2_ps[:, 128 * r:128 * r + 128],
                            lhsT=aggT[:, c, :], rhs=fcnT[r][:, c, :],
                            start=(c == 0), stop=False)
                    nc.tensor.matmul(
                        out2_ps[:, 128 * r:128 * r + 128],
                        lhsT=aggT[0:H, 2, :], rhs=fcnT[r][0:H, 2, :],
                        start=False, stop=True)
                out1_ps = psum_out.tile([128, 2 * 128], F32, tag="outp")
                for r in range(2):
                    for c in range(2):
                        nc.tensor.matmul(
                            out1_ps[:, 128 * r:128 * r + 128],
                            lhsT=xT[:, 2 * gi2 + c, :], rhs=fcxT[r][:, c, :],
                            start=(c == 0), stop=(c == 1))
                outsb = outsbp.tile([128, 2 * O], F32, tag="outsb")
                nc.scalar.activation(outsb[:, 0:O], out1_ps[:], AF.Relu)
                nc.scalar.activation(outsb[:, O:2 * O], out2_ps[:], AF.Relu)
                nc.sync.dma_start(out=out_b[gi2], in_=outsb[:])

            pending = None
            for gi2 in range(n_node_blocks):
                agg_ps = psum_agg.tile([128, DPE + 1], F32, tag="agg")
                for gsub in range(2):
                    gi = 2 * gi2 + gsub
                    # load group: merged natural bf16 tile (cast in-flight)
                    # [neibs 256 | edge 64 | ones 1 | junk 63] per subtile;
                    # junk transposes into unused partitions. Loaded and
                    # transposed in half-group chunks for finer pipelining.
                    nat = natp.tile([128, NB, BLK], BF, tag="nat")
                    tT = ttp.tile([128, 3 * NB, 128], BF, tag="tT")
                    nc.gpsimd.dma_start(out=nat[:, :, 0:D], in_=neibs_g[gi])
                    edgef = edgefp.tile([128, NB, E], F32, tag="edgef")
                    nc.sync.dma_start(out=edgef[:], in_=edge_g[gi])
                    nc.vector.tensor_copy(out=nat[:, :, D:DPE], in_=edgef[:])
                    nc.vector.memset(nat[:, :, DPE:DPE + 1], 1.0)
                    if init_pad:
                        nc.vector.memset(nat[:, :, DPE + 1:BLK], 0)
                    nc.scalar.dma_start(out=tT[:], in_=nat[:], transpose=True)
                    tT_sc = tT[:].rearrange("p (s c) f -> p s c f", c=3)

                    # pre1^T = att2_w1 @ ncat^T, tanh'd on eviction
                    tanh_tiles = []
                    for sg in range(4):
                        pre_ps = psum_pre.tile([H, 512], F32, tag="pre")
                        for c in range(2):
                            nc.tensor.matmul(
                                pre_ps[:], lhsT=a2w1T[:, c, :],
                                rhs=tT_sc[:, 4 * sg:4 * sg + 4, c, :],
                                start=(c == 0), stop=False)
                        for si2 in range(2):
                            s0 = 4 * sg + 2 * si2
                            nc.tensor.matmul(
                                pre_ps[:, 256 * si2:256 * si2 + 256],
                                lhsT=a2w1T[0:H, 2, :],
                                rhs=tT_sc[0:H, s0:s0 + 2, 2, :],
                                start=False, stop=(si2 == 1))
                        ttile = tanhp.tile([H, 512], BF, tag="tanhT")
                        nc.scalar.activation(ttile[:], pre_ps[:], AF.Tanh)
                        tanh_tiles.append(ttile)

                    # pairs scores -> exp -> ws (block-diag), aggregation
                    for gh in range(2):
                        nodes0 = gi * 64 + 32 * gh
                        pairs_ps = psum_pairs.tile([128, 256], F32, tag="pairs")
                        for si8 in range(8):
                            s = 8 * gh + si8
                            nc.tensor.matmul(
                                pairs_ps[:, 32 * si8:32 * si8 + 32],
                                lhsT=tanh_tiles[s // 4][:, 128 * (s % 4):128 * (s % 4) + 128],
                                rhs=zT[:, nodes0:nodes0 + 32],
                                start=True, stop=True)
                        expt = exppl.tile([128, 256], BF, tag="exp")
                        nc.scalar.activation(expt[:], pairs_ps[:], AF.Exp, scale=INV_SQRT_H)
                        strip = (2 * gi + gh) % 4
                        for si8 in range(8):
                            s = 8 * gh + si8
                            S = gi * NB + s
                            ws = wsp.tile([128, K], BF, tag="ws")
                            nc.vector.scalar_tensor_tensor(
                                out=ws[:], in0=expt[:, 32 * si8:32 * si8 + 32],
                                scalar=emT[:, S % 4, (S // 4):(S // 4) + 1],
                                in1=sel8[:, si8, :], op0=MULT, op1=MULT)
                            nc.tensor.matmul(
                                agg_ps[32 * strip:32 * strip + 32, :],
                                lhsT=ws[:], rhs=nat[:, s, 0:DPE + 1],
                                start=(si8 == 0), stop=(si8 == 7),
                                tile_position=(0, 32 * strip))

                if pending is not None:
                    emit_projections(*pending)
                pending = (gi2, agg_ps)
            if pending is not None:
                emit_projections(*pending)

    return nc


_CACHE = {}


def _get_compiled():
    if "nc" not in _CACHE:
        nc = bacc.Bacc("TRN2", target_bir_lowering=False, debug=False)
        build_kernel(nc)
        nc.compile()
        _CACHE["nc"] = nc
    return _CACHE["nc"]


def make_in_maps(x, neibs, edge_emb, mask, att_w1, att_w2, att2_w1, att2_w2,
                 fcx_w, fcn_w):
    x = np.ascontiguousarray(np.asarray(x, np.float32))
    neibs = np.ascontiguousarray(np.asarray(neibs, np.float32))
    edge_emb = np.ascontiguousarray(np.asarray(edge_emb, np.float32))
    mask = np.ascontiguousarray(np.asarray(mask, np.float32))
    w = {
        "att_w1": np.ascontiguousarray(np.asarray(att_w1, np.float32)),
        "att_w2": np.ascontiguousarray(np.asarray(att_w2, np.float32)),
        "att2_w1": np.ascontiguousarray(np.asarray(att2_w1, np.float32)),
        "att2_w2": np.ascontiguousarray(np.asarray(att2_w2, np.float32)),
        "fcx_w": np.ascontiguousarray(np.asarray(fcx_w, np.float32)),
        "fcn_w": np.ascontiguousarray(np.asarray(fcn_w, np.float32)),
    }
    in_maps = []
    for i in range(NCORES):
        in_maps.append({
            "x": x[i * NL:(i + 1) * NL],
            "neibs": neibs[i * ROWS:(i + 1) * ROWS],
            "edge_emb": edge_emb[i * ROWS:(i + 1) * ROWS],
            "mask": mask[i * NL:(i + 1) * NL],
            **w,
        })
    return in_maps


def run(in_maps, trace=False):
    nc = _get_compiled()
    return run_bass_kernel_spmd(nc, in_maps, list(range(NCORES)), trace=trace)


def kernel(x, neibs, edge_emb, mask, att_w1, att_w2, att2_w1, att2_w2,
           fcx_w, fcn_w):
    in_maps = make_in_maps(x, neibs, edge_emb, mask, att_w1, att_w2,
                           att2_w1, att2_w2, fcx_w, fcn_w)
    res = run(in_maps)
    outs = [np.asarray(res.results[i]["out"], np.float32) for i in range(NCORES)]
    return np.concatenate(outs, axis=0)
